# revision 1
# baseline (speedup 1.0000x reference)
"""Trainium2 Bass kernel for HardQuadRadiusTripletLoss.

Computes: per-keypoint dense correlation (2048x256 @ 256x3600 per image),
geometric radius masking (cells whose center is within 8px of the warped
keypoint), top-4 hard negatives, positive-cell similarity, and the
squared-hinge triplet loss reduced to a scalar.

Sharding: data-parallel over batch B=8 -> one image per NeuronCore.

Device pipeline per 128-keypoint tile (16 tiles/core), per 450-col chunk (8):
  PE  : d2m64 = [dy^2|dx^2|1]^T @ bpat      (f32r matmul -> dist2 - 64 in PSUM)
  ACT : u = relu(-K*(d2m64))                (K=2^20; f32r out; 0 outside mask)
  PE  : sim  = kp1_descT.T @ desc2          (f32r, 2 k-passes, PSUM)
        sim += (-I) @ u                     (neg-identity matmul applies mask)
  DVE : chunk top-8 = vector.max(sim_psum)  -> merge buffer
Per tile: DVE merge max over 8x8 chunk maxima -> top-8; indirect row-gather of
desc2T[flat_idx] + fused dot (scalar_tensor_tensor accum) -> positive sim.
Host: input transposes / coordinate prep, final relu(neg-pos+1)^2 mean.
"""

import sys

if "/opt/trn_rl_repo" not in sys.path:
    sys.path.insert(0, "/opt/trn_rl_repo")

import numpy as np

B, N, C, H, W = 8, 2048, 256, 60, 60
HW = H * W            # 3600
GRID = 8.0
NTILE = N // 128      # 16
NCHUNK = 8
CH = HW // NCHUNK     # 450
KPEN = float(2 ** 20)

_NC_CACHE = {}


def _build_nc():
    from concourse import bacc, mybir, bass
    import concourse.tile as tile

    nc = bacc.Bacc("TRN2", target_bir_lowering=False, debug=False)
    f32 = mybir.dt.float32
    f32r = mybir.dt.float32r
    i32 = mybir.dt.int32
    Alu = mybir.AluOpType
    Act = mybir.ActivationFunctionType

    d_desc2f = nc.dram_tensor("desc2f", (C, HW), f32, kind="ExternalInput").ap()
    d_desc2T = nc.dram_tensor("desc2T", (HW, C), f32, kind="ExternalInput").ap()
    d_kpT = nc.dram_tensor("kpT", (C, N), f32, kind="ExternalInput").ap()
    d_kpn = nc.dram_tensor("kpn", (N, C), f32, kind="ExternalInput").ap()
    d_dyxT = nc.dram_tensor("dyxT", (121, N), f32, kind="ExternalInput").ap()
    d_bpat = nc.dram_tensor("bpat", (121, HW), f32, kind="ExternalInput").ap()
    d_negid = nc.dram_tensor("negid", (128, 128), f32, kind="ExternalInput").ap()
    d_fidx = nc.dram_tensor("fidx", (N, 1), i32, kind="ExternalInput").ap()
    d_top8 = nc.dram_tensor("top8", (N, 8), f32, kind="ExternalOutput").ap()
    d_pos = nc.dram_tensor("pos", (N, 1), f32, kind="ExternalOutput").ap()

    with tile.TileContext(nc) as tc:
        with (
            tc.tile_pool(name="pers", bufs=1) as pers,
            tc.tile_pool(name="stage", bufs=2) as stage,
            tc.tile_pool(name="upool", bufs=3) as upool,
            tc.tile_pool(name="work", bufs=3) as work,
            tc.tile_pool(name="ps_d", bufs=2, space="PSUM") as ps_d,
            tc.tile_pool(name="ps_s", bufs=4, space="PSUM") as ps_s,
        ):
            # Persistent f32r operands: DMA load then the mandatory
            # f32r-rounding cast (DVE).
            def load_r(nm, dram_ap, shape):
                st = stage.tile(list(shape), f32, tag="stage")
                nc.sync.dma_start(st[:], dram_ap)
                tr = pers.tile(list(shape), f32r, tag=nm)
                nc.vector.tensor_copy(tr[:], st[:])
                return tr

            dyxT = load_r("dyxT", d_dyxT[:], (121, N))
            bp = load_r("bp", d_bpat[:], (121, HW))
            kpT0 = load_r("kpT0", d_kpT[0:128, :], (128, N))
            kpT1 = load_r("kpT1", d_kpT[128:256, :], (128, N))
            rhs0 = load_r("rhs0", d_desc2f[0:128, :], (128, HW))
            rhs1 = load_r("rhs1", d_desc2f[128:256, :], (128, HW))
            negid = load_r("negid", d_negid[:], (128, 128))

            for t in range(NTILE):
                ns = slice(t * 128, (t + 1) * 128)

                # ---- positive similarity path (exact fp32) ----
                kpn_t = work.tile([128, C], f32, tag="kpn")
                nc.sync.dma_start(kpn_t[:], d_kpn[ns, :])
                fidx_t = work.tile([128, 1], i32, tag="fidx")
                nc.sync.dma_start(fidx_t[:], d_fidx[ns, :])
                posd_t = work.tile([128, C], f32, tag="posd")
                nc.gpsimd.indirect_dma_start(
                    out=posd_t[:],
                    out_offset=None,
                    in_=d_desc2T[:],
                    in_offset=bass.IndirectOffsetOnAxis(ap=fidx_t[:, :1], axis=0),
                )
                junk_t = work.tile([128, C], f32, tag="junk")
                pos_t = work.tile([128, 1], f32, tag="pos")
                nc.vector.scalar_tensor_tensor(
                    out=junk_t[:],
                    in0=posd_t[:],
                    scalar=1.0,
                    in1=kpn_t[:],
                    op0=Alu.mult,
                    op1=Alu.mult,
                    accum_out=pos_t[:],
                )
                nc.sync.dma_start(d_pos[ns, :], pos_t[:])

                # ---- dense correlation + mask + chunkwise top8 ----
                m64 = work.tile([128, 64], f32, tag="m64")
                for c in range(NCHUNK):
                    cs = slice(c * CH, (c + 1) * CH)
                    d2 = ps_d.tile([128, CH], f32, tag="d2")
                    nc.tensor.matmul(
                        out=d2[:], lhsT=dyxT[:, ns], rhs=bp[:, cs],
                        start=True, stop=True,
                    )
                    u = upool.tile([128, CH], f32r, tag="u")
                    nc.scalar.activation(
                        out=u[:], in_=d2[:], func=Act.Relu, scale=-KPEN,
                    )
                    sm = ps_s.tile([128, CH], f32, tag="sm")
                    nc.tensor.matmul(
                        out=sm[:], lhsT=kpT0[:, ns], rhs=rhs0[:, cs],
                        start=True, stop=False,
                    )
                    nc.tensor.matmul(
                        out=sm[:], lhsT=kpT1[:, ns], rhs=rhs1[:, cs],
                        start=False, stop=False,
                    )
                    nc.tensor.matmul(
                        out=sm[:], lhsT=negid[:], rhs=u[:],
                        start=False, stop=True,
                    )
                    nc.vector.max(out=m64[:, c * 8:(c + 1) * 8], in_=sm[:])

                top8_t = work.tile([128, 8], f32, tag="top8")
                nc.vector.max(out=top8_t[:], in_=m64[:])
                nc.sync.dma_start(d_top8[ns, :], top8_t[:])

    nc.compile()
    return nc


def get_nc():
    if "nc" not in _NC_CACHE:
        _NC_CACHE["nc"] = _build_nc()
    return _NC_CACHE["nc"]


def make_in_maps(w_kp1, kp1_desc, desc2):
    yc = ((np.arange(H, dtype=np.float32) + np.float32(0.5)) * np.float32(GRID))
    bpat = np.zeros((121, HW), np.float32)
    for h in range(H):
        bpat[h, h * W:(h + 1) * W] = 1.0
    for w in range(W):
        bpat[60 + w, w::W] = 1.0
    bpat[120, :] = -64.0
    negid = -np.eye(128, dtype=np.float32)

    in_maps = []
    for b in range(B):
        wb = np.asarray(w_kp1[b], dtype=np.float32)
        cy = np.clip(np.floor(wb[:, 0] / np.float32(GRID)).astype(np.int32), 0, H - 1)
        cx = np.clip(np.floor(wb[:, 1] / np.float32(GRID)).astype(np.int32), 0, W - 1)
        fidx = (cy * W + cx).astype(np.int32).reshape(N, 1)
        dy = wb[:, 0:1] - yc[None, :]
        dx = wb[:, 1:2] - yc[None, :]
        dyxT = np.empty((121, N), np.float32)
        dyxT[0:60] = (dy * dy).T
        dyxT[60:120] = (dx * dx).T
        dyxT[120] = 1.0
        kpd = np.ascontiguousarray(np.asarray(kp1_desc[b], dtype=np.float32))
        d2f = np.ascontiguousarray(np.asarray(desc2[b], dtype=np.float32).reshape(C, HW))
        in_maps.append({
            "desc2f": d2f,
            "desc2T": np.ascontiguousarray(d2f.T),
            "kpT": np.ascontiguousarray(kpd.T),
            "kpn": kpd,
            "dyxT": np.ascontiguousarray(dyxT),
            "bpat": bpat,
            "negid": negid,
            "fidx": fidx,
        })
    return in_maps


def finish_loss(results):
    total = 0.0
    for b in range(B):
        out = results[b]
        neg4 = out["top8"][:, :4].astype(np.float64)
        pos = out["pos"].astype(np.float64)
        t = np.maximum(neg4 - pos + 1.0, 0.0)
        total += float((t * t).sum())
    return np.asarray(np.float32(total / (B * N * 4)))


def kernel(kp1, w_kp1, kp1_desc, desc2, homo12):
    from concourse.bass_utils import run_bass_kernel_spmd

    nc = get_nc()
    in_maps = make_in_maps(w_kp1, kp1_desc, desc2)
    res = run_bass_kernel_spmd(nc, in_maps, core_ids=list(range(B)))
    return finish_loss(res.results)



# revision 2
# speedup vs baseline: 2.3493x; 2.3493x over previous
"""Trainium2 Bass kernel for HardQuadRadiusTripletLoss.

Device computes, per image (one per NeuronCore), the dense keypoint/target
correlation (2048x256 @ 256x3600) with fp8e4m3 DoubleRow matmuls (K=256 in a
single PE pass at 0.5 cycles/row) and the UNMASKED per-keypoint top-8, split
across two paths to balance engines:
  - chunks 6-7 (cells 2700:3600): DVE max8 directly on a strided 2-bank PSUM
    access pattern -> top8a (f16).
  - chunks 0-5 (cells 0:2700): ACT Identity moves f32 PSUM -> f16 SBUF (two
    3-bank strided moves), DVE folds pairwise (tensor_tensor max, 2x f16
    mode) 2700->1350->675->338, then max8 -> top8b (f16).

Host does everything cheap/small: fp8 input packing, the positive-cell
similarity (exact fp32), the grid-radius mask (recomputes the <=9 candidate
masked-cell sims from the same fp8 data and removes matching values from the
device top-16 by tolerance), and the final squared-hinge loss. Removing the
mask from the device eliminates the d2 matmul + relu + mask matmul of the
previous version (~48us PE + 72us ACT per core).

Sharding: data-parallel over batch B=8 -> one image per core.
"""

import sys

if "/opt/trn_rl_repo" not in sys.path:
    sys.path.insert(0, "/opt/trn_rl_repo")

import numpy as np
import ml_dtypes

B, N, C, H, W = 8, 2048, 256, 60, 60
HW = H * W            # 3600
GRID = 8.0
NTILE = N // 128      # 16
CH = 450              # chunk width (cells); 8 chunks per tile
SCALE = np.float32(16.0)   # descriptor pre-scale before fp8 cast
SCALE2 = np.float32(SCALE * SCALE)

F8 = ml_dtypes.float8_e4m3  # matches mybir.dt.float8e4

_NC_CACHE = {}
_HOST_CTX = {}


def _build_nc():
    from concourse import bacc, mybir
    import concourse.tile as tile

    nc = bacc.Bacc("TRN2", target_bir_lowering=False, debug=False)
    f32 = mybir.dt.float32
    f16 = mybir.dt.float16
    fp8 = mybir.dt.float8e4
    Alu = mybir.AluOpType
    Act = mybir.ActivationFunctionType
    DR = mybir.MatmulPerfMode.DoubleRow

    d_lhsT = nc.dram_tensor("lhsT", (128, 2, N), fp8, kind="ExternalInput").ap()
    d_rhs = nc.dram_tensor("rhs", (128, 2, HW), fp8, kind="ExternalInput").ap()
    d_top8a = nc.dram_tensor("top8a", (N, 8), f16, kind="ExternalOutput").ap()
    d_top8b = nc.dram_tensor("top8b", (N, 8), f16, kind="ExternalOutput").ap()

    with tile.TileContext(nc) as tc:
        with (
            tc.tile_pool(name="pers", bufs=1) as pers,
            tc.tile_pool(name="mv", bufs=2) as mvp,
            tc.tile_pool(name="fold", bufs=2) as fp,
            tc.tile_pool(name="outp", bufs=2) as outp,
            tc.tile_pool(name="psa1", bufs=1, space="PSUM") as psa1,
            tc.tile_pool(name="psa2", bufs=1, space="PSUM") as psa2,
            tc.tile_pool(name="psb", bufs=1, space="PSUM") as psb,
        ):
            lhsT = pers.tile([128, 2, N], fp8, tag="lhsT")
            rhs = pers.tile([128, 2, HW], fp8, tag="rhs")
            nc.sync.dma_start(lhsT[:], d_lhsT[:])
            nc.sync.dma_start(rhs[:], d_rhs[:])

            for t in range(NTILE):
                ns = slice(t * 128, (t + 1) * 128)
                lt = lhsT[:, :, ns]

                pB = psb.tile([128, 2, 512], f32, tag="pB")
                for j, c in enumerate((6, 7)):
                    nc.tensor.matmul(
                        out=pB[:, j, 0:CH], lhsT=lt,
                        rhs=rhs[:, :, c * CH:(c + 1) * CH],
                        start=True, stop=True, perf_mode=DR,
                    )
                pA1 = psa1.tile([128, 3, 512], f32, tag="pA1")
                for c in (0, 1, 2):
                    nc.tensor.matmul(
                        out=pA1[:, c, 0:CH], lhsT=lt,
                        rhs=rhs[:, :, c * CH:(c + 1) * CH],
                        start=True, stop=True, perf_mode=DR,
                    )
                pA2 = psa2.tile([128, 3, 512], f32, tag="pA2")
                for c in (3, 4, 5):
                    nc.tensor.matmul(
                        out=pA2[:, c - 3, 0:CH], lhsT=lt,
                        rhs=rhs[:, :, c * CH:(c + 1) * CH],
                        start=True, stop=True, perf_mode=DR,
                    )

                top8a = outp.tile([128, 8], f16, tag="top8a")
                nc.vector.max(out=top8a[:], in_=pB[:, :, 0:CH])
                nc.sync.dma_start(d_top8a[ns, :], top8a[:])

                mv1 = mvp.tile([128, 3 * CH], f16, tag="mv1")
                nc.scalar.activation(out=mv1[:], in_=pA1[:, :, 0:CH], func=Act.Identity)
                mv2 = mvp.tile([128, 3 * CH], f16, tag="mv2")
                nc.scalar.activation(out=mv2[:], in_=pA2[:, :, 0:CH], func=Act.Identity)

                f1 = fp.tile([128, 3 * CH], f16, tag="f1")
                nc.vector.tensor_tensor(out=f1[:], in0=mv1[:], in1=mv2[:], op=Alu.max)
                f2 = fp.tile([128, 675], f16, tag="f2")
                nc.vector.tensor_tensor(out=f2[:], in0=f1[:, 0:675], in1=f1[:, 675:1350], op=Alu.max)
                f3 = fp.tile([128, 338], f16, tag="f3")
                # element 337 overlaps both halves; harmless for max
                nc.vector.tensor_tensor(out=f3[:], in0=f2[:, 0:338], in1=f2[:, 337:675], op=Alu.max)
                top8b = outp.tile([128, 8], f16, tag="top8b")
                nc.vector.max(out=top8b[:], in_=f3[:])
                nc.sync.dma_start(d_top8b[ns, :], top8b[:])

    nc.compile()
    return nc


def get_nc():
    if "nc" not in _NC_CACHE:
        _NC_CACHE["nc"] = _build_nc()
    return _NC_CACHE["nc"]


def make_in_maps(w_kp1, kp1_desc, desc2):
    in_maps = []
    ctx = {"w_kp1": np.asarray(w_kp1, np.float32),
           "kq8": [], "dq8": []}
    for b in range(B):
        kpd = np.asarray(kp1_desc[b], np.float32)          # N,C
        d2f = np.asarray(desc2[b], np.float32).reshape(C, HW)
        kq8 = (kpd * SCALE).astype(F8)                     # N,C
        dq8 = (d2f * SCALE).astype(F8)                     # C,HW
        ctx["kq8"].append(kq8)
        ctx["dq8"].append(dq8)
        lhsT = np.ascontiguousarray(
            kq8.T.reshape(2, 128, N).transpose(1, 0, 2))   # 128,2,N
        rhs = np.ascontiguousarray(
            dq8.reshape(2, 128, HW).transpose(1, 0, 2))    # 128,2,HW
        in_maps.append({"lhsT": lhsT, "rhs": rhs})
    ctx["kp1_desc"] = np.asarray(kp1_desc, np.float32)
    ctx["desc2"] = np.asarray(desc2, np.float32)
    _HOST_CTX.clear()
    _HOST_CTX.update(ctx)
    return in_maps


def finish_loss(results):
    w_kp1 = _HOST_CTX["w_kp1"]
    kp1_desc = _HOST_CTX["kp1_desc"]
    desc2 = _HOST_CTX["desc2"]

    yc = (np.arange(H, dtype=np.float32) + np.float32(0.5)) * np.float32(GRID)
    offs = np.array([(dy, dx) for dy in (-1, 0, 1) for dx in (-1, 0, 1)], np.int32)

    total = 0.0
    for b in range(B):
        wb = w_kp1[b]
        kpd = kp1_desc[b]
        d2f = desc2[b].reshape(C, HW)

        # positive similarity (exact fp32, mirrors reference)
        cy = np.clip(np.floor(wb[:, 0] / np.float32(GRID)).astype(np.int32), 0, H - 1)
        cx = np.clip(np.floor(wb[:, 1] / np.float32(GRID)).astype(np.int32), 0, W - 1)
        fidx = cy * W + cx
        pos = np.einsum("nc,cn->n", kpd, d2f[:, fidx]).astype(np.float32)

        # masked cells: centers within GRID px of the warped keypoint
        h0 = np.clip(np.round((wb[:, 0] - 4.0) / 8.0).astype(np.int32), 0, H - 1)
        w0 = np.clip(np.round((wb[:, 1] - 4.0) / 8.0).astype(np.int32), 0, W - 1)
        hh = h0[:, None] + offs[None, :, 0]
        ww = w0[:, None] + offs[None, :, 1]
        valid = (hh >= 0) & (hh < H) & (ww >= 0) & (ww < W)
        hhc = np.clip(hh, 0, H - 1)
        wwc = np.clip(ww, 0, W - 1)
        d2c = (wb[:, 0:1] - yc[hhc]) ** 2 + (wb[:, 1:2] - yc[wwc]) ** 2
        masked = valid & (d2c <= np.float32(GRID * GRID))
        midx = hhc * W + wwc                                   # N,9

        # masked-cell sims through the same fp8 pipeline + f16 rounding
        kq = _HOST_CTX["kq8"][b].astype(np.float32)            # N,C (scaled)
        dq = _HOST_CTX["dq8"][b].astype(np.float32)            # C,HW (scaled)
        msim = np.einsum("nc,nkc->nk", kq, dq[:, midx].transpose(1, 2, 0))
        msim = msim.astype(np.float16).astype(np.float32)      # N,9

        ta = results[b]["top8a"].astype(np.float32)            # N,8
        tb = results[b]["top8b"].astype(np.float32)            # N,8
        cand = np.concatenate([ta, tb], axis=1)                # N,16
        cand = -np.sort(-cand, axis=1)

        neg4 = np.empty((N, 4), np.float32)
        # fast path: keypoints with no masked value near the top-8
        thresh = cand[:, 7] - np.float32(0.25)
        any_hit = ((msim >= thresh[:, None]) & masked).any(axis=1)
        neg4[~any_hit] = cand[~any_hit, :4]
        for n in np.nonzero(any_hit)[0]:
            vals = list(cand[n])
            for j in range(9):
                if not masked[n, j]:
                    continue
                m = msim[n, j]
                if m < thresh[n]:
                    continue
                eps = max(0.25, abs(m) * 2.0 ** -8)
                bd, best = 1e9, -1
                for i, v in enumerate(vals):
                    d = abs(v - m)
                    if d < bd:
                        bd, best = d, i
                if best >= 0 and bd <= eps:
                    vals.pop(best)
            while len(vals) < 4:
                vals.append(vals[-1])
            neg4[n] = vals[:4]

        neg4 = neg4 / SCALE2
        t = np.maximum(neg4 - pos[:, None] + np.float32(1.0), 0.0)
        total += float((t.astype(np.float64) ** 2).sum())

    return np.asarray(np.float32(total / (B * N * 4)))


def kernel(kp1, w_kp1, kp1_desc, desc2, homo12):
    from concourse.bass_utils import run_bass_kernel_spmd

    nc = get_nc()
    in_maps = make_in_maps(w_kp1, kp1_desc, desc2)
    res = run_bass_kernel_spmd(nc, in_maps, core_ids=list(range(B)))
    return finish_loss(res.results)


# revision 4
# speedup vs baseline: 2.8135x; 1.1976x over previous
"""Trainium2 Bass kernel for HardQuadRadiusTripletLoss.

Device computes, per image (one per NeuronCore), a PE-folded dense
correlation top-8. Cells are paired (j, j+1800); for each pair the PE
computes M = b + relu(a-b) = max(a, b) directly in PSUM:
  - mm_b:   sims of the b-side cells   (fp8e4m3 DoubleRow, K=256, 0.5 cy/row)
  - mm_d:   D = a-b sims from a host-precomputed difference map (fp8 DR)
  - ACT:    u = relu(D) -> bf16 SBUF   (the only per-element ACT pass)
  - acc:    M += Identity @ u          (bf16 matmul accumulate into mm_b PSUM)
  - DVE:    max8 over 900 pairs (strided 2-bank PSUM read) -> f16 top8
This halves the DVE scan (1800 pairs instead of 3600 cells per keypoint
tile) and eliminates the mask pipeline entirely.

Host does the cheap/small work: fp8 packing, the positive-cell similarity
(exact fp32), and the grid-radius mask: for each of the <=9 candidate masked
cells it recomputes the pair's (b, D) through the same quantized arithmetic,
removes the pair-max from the device top-16 when the masked side won the
pair, and inserts the surviving partner value. Squared-hinge loss in fp64.

Sharding: data-parallel over batch B=8 -> one image per core.
"""

import sys

if "/opt/trn_rl_repo" not in sys.path:
    sys.path.insert(0, "/opt/trn_rl_repo")

import numpy as np
import ml_dtypes

B, N, C, H, W = 8, 2048, 256, 60, 60
HW = H * W            # 3600
P = HW // 2           # 1800 pairs
GRID = 8.0
NTILE = N // 128      # 16
NH = 2 * NTILE        # 32 half-tiles (900 pairs each)
SCALE = np.float32(16.0)
SCALE2 = np.float32(SCALE * SCALE)

F8 = ml_dtypes.float8_e4m3  # matches mybir.dt.float8e4
BF16 = ml_dtypes.bfloat16

_NC_CACHE = {}
_HOST_CTX = {}


def _build_nc():
    from concourse import bacc, mybir
    import concourse.tile as tile

    nc = bacc.Bacc("TRN2", target_bir_lowering=False, debug=False)
    f32 = mybir.dt.float32
    f16 = mybir.dt.float16
    bf16 = mybir.dt.bfloat16
    fp8 = mybir.dt.float8e4
    Act = mybir.ActivationFunctionType
    DR = mybir.MatmulPerfMode.DoubleRow

    d_lhsT = nc.dram_tensor("lhsT", (128, 2, N), fp8, kind="ExternalInput").ap()
    d_rhsb = nc.dram_tensor("rhsb", (128, 2, P), fp8, kind="ExternalInput").ap()
    d_rhsd = nc.dram_tensor("rhsd", (128, 2, P), fp8, kind="ExternalInput").ap()
    d_ident = nc.dram_tensor("ident", (128, 128), bf16, kind="ExternalInput").ap()
    d_top8 = nc.dram_tensor("top8", (N, 16), f16, kind="ExternalOutput").ap()

    with tile.TileContext(nc) as tc:
        with (
            tc.tile_pool(name="pers", bufs=1) as pers,
            tc.tile_pool(name="ub", bufs=3) as ubp,
            tc.tile_pool(name="outp", bufs=3) as outp,
            tc.tile_pool(name="psm", bufs=2, space="PSUM") as psm,
            tc.tile_pool(name="psd", bufs=2, space="PSUM") as psd,
        ):
            # split inputs into independently-DMA'd tiles so the first
            # matmuls start as soon as their slice has landed
            ident = pers.tile([128, 128], bf16, tag="ident")
            nc.sync.dma_start(ident[:], d_ident[:])
            rhsd = [pers.tile([128, 2, 900], fp8, tag=f"rhsd{h}", name=f"rhsd{h}")
                    for h in range(2)]
            rhsb = [pers.tile([128, 2, 900], fp8, tag=f"rhsb{h}", name=f"rhsb{h}")
                    for h in range(2)]
            lhsT = [pers.tile([128, 2, 512], fp8, tag=f"lhsT{q}", name=f"lhsT{q}")
                    for q in range(4)]
            nc.sync.dma_start(lhsT[0][:], d_lhsT[:, :, 0:512])
            for h in range(2):
                nc.sync.dma_start(rhsd[h][:], d_rhsd[:, :, h * 900:(h + 1) * 900])
                nc.sync.dma_start(rhsb[h][:], d_rhsb[:, :, h * 900:(h + 1) * 900])
            for q in range(1, 4):
                nc.sync.dma_start(lhsT[q][:], d_lhsT[:, :, q * 512:(q + 1) * 512])

            def lt(t):
                return lhsT[t // 4][:, :, (t % 4) * 128:(t % 4) * 128 + 128]

            prev = None  # (M-psum tile, u tile, t, h)
            for step in range(NH):
                t, h = step // 2, step % 2
                pd = psd.tile([128, 2, 512], f32, tag="pd")
                for k in range(2):
                    nc.tensor.matmul(
                        out=pd[:, k, 0:450], lhsT=lt(t),
                        rhs=rhsd[h][:, :, k * 450:(k + 1) * 450],
                        start=True, stop=True, perf_mode=DR,
                    )
                pm = psm.tile([128, 2, 512], f32, tag="pm")
                for k in range(2):
                    nc.tensor.matmul(
                        out=pm[:, k, 0:450], lhsT=lt(t),
                        rhs=rhsb[h][:, :, k * 450:(k + 1) * 450],
                        start=True, stop=False, perf_mode=DR,
                    )
                if prev is not None:
                    _finish(nc, tc, outp, d_top8, prev, f16)
                u = ubp.tile([128, 900], bf16, tag="u")
                nc.scalar.activation(out=u[:], in_=pd[:, :, 0:450], func=Act.Relu)
                prev = (pm, u, ident, t, h)
            _finish(nc, tc, outp, d_top8, prev, f16)

    nc.compile()
    return nc


def _finish(nc, tc, outp, d_top8, prev, f16):
    pm, u, ident, t, h = prev
    for k in range(2):
        nc.tensor.matmul(
            out=pm[:, k, 0:450], lhsT=ident[:],
            rhs=u[:, k * 450:(k + 1) * 450],
            start=False, stop=True, skip_group_check=True,
        )
    top8 = outp.tile([128, 8], f16, tag="top8")
    nc.vector.max(out=top8[:], in_=pm[:, :, 0:450])
    ns = slice(t * 128, (t + 1) * 128)
    nc.sync.dma_start(d_top8[ns, h * 8:(h + 1) * 8], top8[:])


def get_nc():
    if "nc" not in _NC_CACHE:
        _NC_CACHE["nc"] = _build_nc()
    return _NC_CACHE["nc"]


def make_in_maps(w_kp1, kp1_desc, desc2):
    in_maps = []
    ctx = {"w_kp1": np.asarray(w_kp1, np.float32),
           "kq8": [], "dqb8": [], "dqd8": []}
    ident = np.eye(128, dtype=BF16)
    for b in range(B):
        kpd = np.asarray(kp1_desc[b], np.float32)          # N,C
        d2f = np.asarray(desc2[b], np.float32).reshape(C, HW)
        kq8 = (kpd * SCALE).astype(F8)                     # N,C
        dqb8 = (d2f[:, P:] * SCALE).astype(F8)             # C,P
        dqd8 = ((d2f[:, :P] - d2f[:, P:]) * SCALE).astype(F8)
        ctx["kq8"].append(kq8)
        ctx["dqb8"].append(dqb8)
        ctx["dqd8"].append(dqd8)
        lhsT = np.ascontiguousarray(
            kq8.T.reshape(2, 128, N).transpose(1, 0, 2))   # 128,2,N
        rhsb = np.ascontiguousarray(
            dqb8.reshape(2, 128, P).transpose(1, 0, 2))    # 128,2,P
        rhsd = np.ascontiguousarray(
            dqd8.reshape(2, 128, P).transpose(1, 0, 2))
        in_maps.append({"lhsT": lhsT, "rhsb": rhsb, "rhsd": rhsd, "ident": ident})
    ctx["kp1_desc"] = np.asarray(kp1_desc, np.float32)
    ctx["desc2"] = np.asarray(desc2, np.float32)
    _HOST_CTX.clear()
    _HOST_CTX.update(ctx)
    return in_maps


def finish_loss(results):
    w_kp1 = _HOST_CTX["w_kp1"]
    kp1_desc = _HOST_CTX["kp1_desc"]
    desc2 = _HOST_CTX["desc2"]

    yc = (np.arange(H, dtype=np.float32) + np.float32(0.5)) * np.float32(GRID)
    offs = np.array([(dy, dx) for dy in (-1, 0, 1) for dx in (-1, 0, 1)], np.int32)

    total = 0.0
    for b in range(B):
        wb = w_kp1[b]
        kpd = kp1_desc[b]
        d2f = desc2[b].reshape(C, HW)

        # positive similarity (exact fp32, mirrors reference)
        cy = np.clip(np.floor(wb[:, 0] / np.float32(GRID)).astype(np.int32), 0, H - 1)
        cx = np.clip(np.floor(wb[:, 1] / np.float32(GRID)).astype(np.int32), 0, W - 1)
        fidx = cy * W + cx
        pos = np.einsum("nc,cn->n", kpd, d2f[:, fidx]).astype(np.float32)

        # masked cells: centers within GRID px of the warped keypoint
        h0 = np.clip(np.round((wb[:, 0] - 4.0) / 8.0).astype(np.int32), 0, H - 1)
        w0 = np.clip(np.round((wb[:, 1] - 4.0) / 8.0).astype(np.int32), 0, W - 1)
        hh = h0[:, None] + offs[None, :, 0]
        ww = w0[:, None] + offs[None, :, 1]
        valid = (hh >= 0) & (hh < H) & (ww >= 0) & (ww < W)
        hhc = np.clip(hh, 0, H - 1)
        wwc = np.clip(ww, 0, W - 1)
        d2c = (wb[:, 0:1] - yc[hhc]) ** 2 + (wb[:, 1:2] - yc[wwc]) ** 2
        masked = valid & (d2c <= np.float32(GRID * GRID))
        midx = hhc * W + wwc                                   # N,9

        cand = results[b]["top8"].astype(np.float32)           # N,16
        cand = -np.sort(-cand, axis=1)

        # masked pairs through the same quantized arithmetic
        kq = _HOST_CTX["kq8"][b].astype(np.float32)
        dqb = _HOST_CTX["dqb8"][b].astype(np.float32)
        dqd = _HOST_CTX["dqd8"][b].astype(np.float32)
        pj = np.where(midx < P, midx, midx - P)
        is_a = midx < P
        bq = np.einsum("nc,nkc->nk", kq, dqb[:, pj].transpose(1, 2, 0))
        Dq = np.einsum("nc,nkc->nk", kq, dqd[:, pj].transpose(1, 2, 0))
        uq = np.maximum(Dq, 0.0).astype(BF16).astype(np.float32)
        Mq = (bq + uq).astype(np.float16).astype(np.float32)

        thresh = cand[:, 7] - np.float32(0.5)
        hit = masked & (Mq >= thresh[:, None]) & np.where(is_a, Dq > 0, Dq <= 0)
        any_hit = hit.any(axis=1)

        neg4 = np.empty((N, 4), np.float32)
        neg4[~any_hit] = cand[~any_hit, :4]
        for n in np.nonzero(any_hit)[0]:
            vals = list(cand[n])
            for j in range(9):
                if not hit[n, j]:
                    continue
                m = Mq[n, j]
                eps = max(0.5, abs(m) * 2.0 ** -8)
                bd, best = 1e9, -1
                for i, v in enumerate(vals):
                    d = abs(v - m)
                    if d < bd:
                        bd, best = d, i
                if best >= 0 and bd <= eps:
                    vals.pop(best)
                    ins = bq[n, j] if is_a[n, j] else bq[n, j] + Dq[n, j]
                    vals.append(np.float32(ins))
            vals = sorted(vals, reverse=True)
            neg4[n] = vals[:4]

        neg4 = neg4 / SCALE2
        t = np.maximum(neg4 - pos[:, None] + np.float32(1.0), 0.0)
        total += float((t.astype(np.float64) ** 2).sum())

    return np.asarray(np.float32(total / (B * N * 4)))


def kernel(kp1, w_kp1, kp1_desc, desc2, homo12):
    from concourse.bass_utils import run_bass_kernel_spmd

    nc = get_nc()
    in_maps = make_in_maps(w_kp1, kp1_desc, desc2)
    res = run_bass_kernel_spmd(nc, in_maps, core_ids=list(range(B)))
    return finish_loss(res.results)


# revision 8
# speedup vs baseline: 2.8982x; 1.0301x over previous
"""Trainium2 Bass kernel for HardQuadRadiusTripletLoss.

Device computes, per image (one per NeuronCore), a PE-folded dense
correlation top-8. Cells are paired (j, j+1800); for each pair the PE
computes M = b + relu(a-b) = max(a, b) directly in PSUM:
  - mm_b:   sims of the b-side cells   (fp8e4m3 DoubleRow, K=256, 0.5 cy/row)
  - mm_d:   D = a-b sims from a host-precomputed difference map (fp8 DR)
  - ACT:    u = relu(D) -> bf16 SBUF   (the only per-element ACT pass)
  - acc:    M += Identity @ u          (bf16 matmul accumulate into mm_b PSUM)
  - DVE:    max8 over 900 pairs (strided 2-bank PSUM read) -> f16 top8
This halves the DVE scan (1800 pairs instead of 3600 cells per keypoint
tile) and eliminates the mask pipeline entirely.

Host does the cheap/small work: fp8 packing, the positive-cell similarity
(exact fp32), and the grid-radius mask: for each of the <=9 candidate masked
cells it recomputes the pair's (b, D) through the same quantized arithmetic,
removes the pair-max from the device top-16 when the masked side won the
pair, and inserts the surviving partner value. Squared-hinge loss in fp64.

Sharding: data-parallel over batch B=8 -> one image per core.
"""

import sys

if "/opt/trn_rl_repo" not in sys.path:
    sys.path.insert(0, "/opt/trn_rl_repo")

import numpy as np
import ml_dtypes

B, N, C, H, W = 8, 2048, 256, 60, 60
HW = H * W            # 3600
P = HW // 2           # 1800 pairs
GRID = 8.0
NTILE = N // 128      # 16
NH = 2 * NTILE        # 32 half-tiles (900 pairs each)
SCALE = np.float32(16.0)
SCALE2 = np.float32(SCALE * SCALE)

F8 = ml_dtypes.float8_e4m3  # matches mybir.dt.float8e4
BF16 = ml_dtypes.bfloat16

_NC_CACHE = {}
_HOST_CTX = {}


def _build_nc():
    from concourse import bacc, mybir
    import concourse.tile as tile

    nc = bacc.Bacc("TRN2", target_bir_lowering=False, debug=False)
    f32 = mybir.dt.float32
    f16 = mybir.dt.float16
    bf16 = mybir.dt.bfloat16
    fp8 = mybir.dt.float8e4
    Act = mybir.ActivationFunctionType
    DR = mybir.MatmulPerfMode.DoubleRow

    d_lhsT = nc.dram_tensor("lhsT", (128, 2, N), fp8, kind="ExternalInput").ap()
    d_rhsb = nc.dram_tensor("rhsb", (128, 2, P), fp8, kind="ExternalInput").ap()
    d_rhsd = nc.dram_tensor("rhsd", (128, 2, P), fp8, kind="ExternalInput").ap()
    d_ident = nc.dram_tensor("ident", (128, 128), bf16, kind="ExternalInput").ap()
    d_top8 = nc.dram_tensor("top8", (N, 16), f16, kind="ExternalOutput").ap()

    with tile.TileContext(nc) as tc:
        with (
            tc.tile_pool(name="pers", bufs=1) as pers,
            tc.tile_pool(name="ub", bufs=4) as ubp,
            tc.tile_pool(name="outp", bufs=6) as outp,
            tc.tile_pool(name="psm", bufs=2, space="PSUM") as psm,
            tc.tile_pool(name="psd", bufs=2, space="PSUM") as psd,
        ):
            # split inputs into independently-DMA'd tiles so the first
            # matmuls start as soon as their slice has landed; late pieces
            # go out on the DVE queue (idle until the first Max anyway)
            ident = pers.tile([128, 128], bf16, tag="ident")
            rhsd = [pers.tile([128, 2, 450], fp8, tag=f"rhsd{p}", name=f"rhsd{p}")
                    for p in range(4)]
            rhsb = [pers.tile([128, 2, 450], fp8, tag=f"rhsb{p}", name=f"rhsb{p}")
                    for p in range(4)]
            lhsT = [pers.tile([128, 2, 512], fp8, tag=f"lhsT{q}", name=f"lhsT{q}")
                    for q in range(4)]
            nc.sync.dma_start(rhsd[0][:], d_rhsd[:, :, 0:450])
            nc.sync.dma_start(lhsT[0][:], d_lhsT[:, :, 0:512])
            nc.sync.dma_start(rhsb[0][:], d_rhsb[:, :, 0:450])
            nc.sync.dma_start(rhsd[1][:], d_rhsd[:, :, 450:900])
            nc.sync.dma_start(rhsb[1][:], d_rhsb[:, :, 450:900])
            nc.sync.dma_start(ident[:], d_ident[:])
            for p in (2, 3):
                nc.gpsimd.dma_start(rhsd[p][:], d_rhsd[:, :, p * 450:(p + 1) * 450])
                nc.gpsimd.dma_start(rhsb[p][:], d_rhsb[:, :, p * 450:(p + 1) * 450])
            for q in range(1, 4):
                nc.sync.dma_start(lhsT[q][:], d_lhsT[:, :, q * 512:(q + 1) * 512])

            def lt(t):
                return lhsT[t // 4][:, :, (t % 4) * 128:(t % 4) * 128 + 128]

            prev = None  # (M-psum tile, u tile, t, h)
            for step in range(NH):
                t, h = step // 2, step % 2
                pd = psd.tile([128, 2, 512], f32, tag="pd")
                for k in range(2):
                    nc.tensor.matmul(
                        out=pd[:, k, 0:450], lhsT=lt(t),
                        rhs=rhsd[2 * h + k][:],
                        start=True, stop=True, perf_mode=DR,
                    )
                pm = psm.tile([128, 2, 512], f32, tag="pm")
                for k in range(2):
                    nc.tensor.matmul(
                        out=pm[:, k, 0:450], lhsT=lt(t),
                        rhs=rhsb[2 * h + k][:],
                        start=True, stop=False, perf_mode=DR,
                    )
                if prev is not None:
                    _finish(nc, tc, outp, d_top8, prev, f16)
                u = ubp.tile([128, 900], bf16, tag="u")
                nc.scalar.activation(out=u[:], in_=pd[:, :, 0:450], func=Act.Relu)
                prev = (pm, u, ident, t, h)
            _finish(nc, tc, outp, d_top8, prev, f16)

    nc.compile()
    return nc


def _finish(nc, tc, outp, d_top8, prev, f16):
    pm, u, ident, t, h = prev
    for k in range(2):
        nc.tensor.matmul(
            out=pm[:, k, 0:450], lhsT=ident[:],
            rhs=u[:, k * 450:(k + 1) * 450],
            start=False, stop=True, skip_group_check=True,
        )
    top8 = outp.tile([128, 8], f16, tag="top8")
    nc.vector.max(out=top8[:], in_=pm[:, :, 0:450])
    ns = slice(t * 128, (t + 1) * 128)
    nc.sync.dma_start(d_top8[ns, h * 8:(h + 1) * 8], top8[:])


def get_nc():
    if "nc" not in _NC_CACHE:
        _NC_CACHE["nc"] = _build_nc()
    return _NC_CACHE["nc"]


def make_in_maps(w_kp1, kp1_desc, desc2):
    in_maps = []
    ctx = {"w_kp1": np.asarray(w_kp1, np.float32),
           "kq8": [], "dqb8": [], "dqd8": []}
    ident = np.eye(128, dtype=BF16)
    for b in range(B):
        kpd = np.asarray(kp1_desc[b], np.float32)          # N,C
        d2f = np.asarray(desc2[b], np.float32).reshape(C, HW)
        kq8 = (kpd * SCALE).astype(F8)                     # N,C
        dqb8 = (d2f[:, P:] * SCALE).astype(F8)             # C,P
        dqd8 = ((d2f[:, :P] - d2f[:, P:]) * SCALE).astype(F8)
        ctx["kq8"].append(kq8)
        ctx["dqb8"].append(dqb8)
        ctx["dqd8"].append(dqd8)
        lhsT = np.ascontiguousarray(
            kq8.T.reshape(2, 128, N).transpose(1, 0, 2))   # 128,2,N
        rhsb = np.ascontiguousarray(
            dqb8.reshape(2, 128, P).transpose(1, 0, 2))    # 128,2,P
        rhsd = np.ascontiguousarray(
            dqd8.reshape(2, 128, P).transpose(1, 0, 2))
        in_maps.append({"lhsT": lhsT, "rhsb": rhsb, "rhsd": rhsd, "ident": ident})
    ctx["kp1_desc"] = np.asarray(kp1_desc, np.float32)
    ctx["desc2"] = np.asarray(desc2, np.float32)
    _HOST_CTX.clear()
    _HOST_CTX.update(ctx)
    return in_maps


def finish_loss(results):
    w_kp1 = _HOST_CTX["w_kp1"]
    kp1_desc = _HOST_CTX["kp1_desc"]
    desc2 = _HOST_CTX["desc2"]

    yc = (np.arange(H, dtype=np.float32) + np.float32(0.5)) * np.float32(GRID)
    offs = np.array([(dy, dx) for dy in (-1, 0, 1) for dx in (-1, 0, 1)], np.int32)

    total = 0.0
    for b in range(B):
        wb = w_kp1[b]
        kpd = kp1_desc[b]
        d2f = desc2[b].reshape(C, HW)

        # positive similarity (exact fp32, mirrors reference)
        cy = np.clip(np.floor(wb[:, 0] / np.float32(GRID)).astype(np.int32), 0, H - 1)
        cx = np.clip(np.floor(wb[:, 1] / np.float32(GRID)).astype(np.int32), 0, W - 1)
        fidx = cy * W + cx
        pos = np.einsum("nc,cn->n", kpd, d2f[:, fidx]).astype(np.float32)

        # masked cells: centers within GRID px of the warped keypoint
        h0 = np.clip(np.round((wb[:, 0] - 4.0) / 8.0).astype(np.int32), 0, H - 1)
        w0 = np.clip(np.round((wb[:, 1] - 4.0) / 8.0).astype(np.int32), 0, W - 1)
        hh = h0[:, None] + offs[None, :, 0]
        ww = w0[:, None] + offs[None, :, 1]
        valid = (hh >= 0) & (hh < H) & (ww >= 0) & (ww < W)
        hhc = np.clip(hh, 0, H - 1)
        wwc = np.clip(ww, 0, W - 1)
        d2c = (wb[:, 0:1] - yc[hhc]) ** 2 + (wb[:, 1:2] - yc[wwc]) ** 2
        masked = valid & (d2c <= np.float32(GRID * GRID))
        midx = hhc * W + wwc                                   # N,9

        cand = results[b]["top8"].astype(np.float32)           # N,16
        cand = -np.sort(-cand, axis=1)

        # masked pairs through the same quantized arithmetic
        kq = _HOST_CTX["kq8"][b].astype(np.float32)
        dqb = _HOST_CTX["dqb8"][b].astype(np.float32)
        dqd = _HOST_CTX["dqd8"][b].astype(np.float32)
        pj = np.where(midx < P, midx, midx - P)
        is_a = midx < P
        bq = np.einsum("nc,nkc->nk", kq, dqb[:, pj].transpose(1, 2, 0))
        Dq = np.einsum("nc,nkc->nk", kq, dqd[:, pj].transpose(1, 2, 0))
        uq = np.maximum(Dq, 0.0).astype(BF16).astype(np.float32)
        Mq = (bq + uq).astype(np.float16).astype(np.float32)

        thresh = cand[:, 7] - np.float32(0.5)
        hit = masked & (Mq >= thresh[:, None]) & np.where(is_a, Dq > 0, Dq <= 0)
        any_hit = hit.any(axis=1)

        neg4 = np.empty((N, 4), np.float32)
        neg4[~any_hit] = cand[~any_hit, :4]
        for n in np.nonzero(any_hit)[0]:
            vals = list(cand[n])
            for j in range(9):
                if not hit[n, j]:
                    continue
                m = Mq[n, j]
                eps = max(0.5, abs(m) * 2.0 ** -8)
                bd, best = 1e9, -1
                for i, v in enumerate(vals):
                    d = abs(v - m)
                    if d < bd:
                        bd, best = d, i
                if best >= 0 and bd <= eps:
                    vals.pop(best)
                    ins = bq[n, j] if is_a[n, j] else bq[n, j] + Dq[n, j]
                    vals.append(np.float32(ins))
            vals = sorted(vals, reverse=True)
            neg4[n] = vals[:4]

        neg4 = neg4 / SCALE2
        t = np.maximum(neg4 - pos[:, None] + np.float32(1.0), 0.0)
        total += float((t.astype(np.float64) ** 2).sum())

    return np.asarray(np.float32(total / (B * N * 4)))


def kernel(kp1, w_kp1, kp1_desc, desc2, homo12):
    from concourse.bass_utils import run_bass_kernel_spmd

    nc = get_nc()
    in_maps = make_in_maps(w_kp1, kp1_desc, desc2)
    res = run_bass_kernel_spmd(nc, in_maps, core_ids=list(range(B)))
    return finish_loss(res.results)
